# revision 15
# baseline (speedup 1.0000x reference)
"""DynamicFilter Trainium2 kernel.

Computation (per sample b):
    h  = tanh(query @ W1.T + b1)                      [B, 256]
    cw = (h @ W2.T + b2).reshape(B, C=32, K=31)       per-sample conv weights
    x[b,t,c] = sum_k cw[b,c,k] * pad(prev_attn)[b, t+k]
    out[b,t,o] = sum_c Wfc[o,c] x[b,t,c] + bfc[o]

Key algebraic fusion: fold the fc into the conv,
    Weff[b,o,k] = sum_c Wfc[o,c] cw[b,c,k]            [B, 128, 31]
    out[b,t,o]  = sum_k Weff[b,o,k] pad(prev_attn)[b, t+k] + bfc[o]
so the T-sized work is ONE matmul per (sample, 512-wide t-chunk):
    psum[128 o, 512 t] = WeffT_b[32 k, 128 o].T @ windows[32 k, 512 t]
with the windows operand streamed from SBUF tiles holding 31 shifted
replicas of each padded row plus a row of ones; the matching 32nd
stationary row holds bfc, so PSUM accumulates conv + bias exactly in
fp32 and the psum->sbuf drain is a plain dtype-narrowing copy.

The whole T-sized data path runs in bf16 (the correctness gate is
rel_err < 2e-2; bf16 rounding costs ~4e-3): replicas, matmul operands
and the output stream are all bf16, halving HBM traffic -- the f32
profile showed all 16 SDMA engines pegged at the ~350 GB/s HBM
roofline.  PSUM accumulation stays fp32.

Main-loop discipline: the PE p-state ramps to 2.4 GHz only after ~3us
of CONTINUOUS execution (it idles at 1.2 GHz otherwise), so the 64
conv matmuls are kept gap-free: 5 PSUM banks round-robin, the drain
copies rotate over vector/scalar/gpsimd so no single engine falls
behind, and all out-DMA dispatches live on the sync queue.

Sharding: data-parallel over batch. 64 samples / 8 cores = 8 per core.
Weights replicated. Output written bf16 [b, o, t] in [128, 2048] tiles
(4 KB contiguous runs); host upcasts and returns a transposed view
[B, T, O] in f32.
"""

import sys

import numpy as np

if "/opt/trn_rl_repo" not in sys.path:
    sys.path.insert(0, "/opt/trn_rl_repo")

from contextlib import ExitStack

import ml_dtypes

import concourse.bass as bass
import concourse.mybir as mybir
import concourse.tile as tile
from concourse import bacc
from concourse.ap import AP
from concourse.bass_utils import run_bass_kernel_spmd
from concourse.masks import make_identity

# Problem shapes (hardcoded per contract).
B, T = 64, 4096
D, H = 1024, 256
C, K, O = 32, 31, 128
KB = K + 1  # conv taps + the folded-bias ones row
PAD = (K - 1) // 2  # 15
NCORES = 8
BPC = B // NCORES  # 8 samples per core
TCH = 512  # t-chunk (matmul moving free dim, one PSUM bank)
NT = T // TCH  # 8 chunks per sample
OCH = 2048  # out-tile column width (4 KB bf16 runs)
PCH = 1024  # replica chunk width (2 KB bf16 runs)
GROUPS = [(0, 3), (3, 3), (6, 2)]  # (first sample, count) per replica tile

F32 = mybir.dt.float32
BF16 = mybir.dt.bfloat16
AF = mybir.ActivationFunctionType
BF16NP = ml_dtypes.bfloat16

_CACHED = {}


def _build_nc():
    nc = bacc.Bacc(
        "TRN2", target_bir_lowering=False, debug=False, num_devices=NCORES
    )

    # host-prepacked layouts: single contiguous DMAs into the exact SBUF
    # images (descriptor count on the HWDGE rings is a scarce resource)
    qT_h = nc.dram_tensor("qtp", [128, 8 * BPC], BF16, kind="ExternalInput")
    rep_h = nc.dram_tensor("paRep", [len(GROUPS), 96, T], BF16,
                           kind="ExternalInput")
    w1t_h = nc.dram_tensor("w1tp", [128, 8 * H], BF16, kind="ExternalInput")
    b1_h = nc.dram_tensor("b1p", [128, 2], F32, kind="ExternalInput")
    w2t_h = nc.dram_tensor("w2tp", [128, 2 * C * K], BF16,
                           kind="ExternalInput")
    b2_h = nc.dram_tensor("b2", [C * K], BF16, kind="ExternalInput")
    wfct_h = nc.dram_tensor("wfct", [C, O], F32, kind="ExternalInput")
    bfcr_h = nc.dram_tensor("bfcr", [3, O], BF16, kind="ExternalInput")
    out_h = nc.dram_tensor("out", [BPC, O, T], BF16, kind="ExternalOutput")

    with tile.TileContext(nc) as tc:
        _emit(tc, qT_h, rep_h, w1t_h, b1_h, w2t_h, b2_h, wfct_h, bfcr_h, out_h)

    nc.compile()
    return nc


def _emit(tc, qT_h, rep_h, w1t_h, b1_h, w2t_h, b2_h, wfct_h, bfcr_h, out_h):
    nc = tc.nc
    with ExitStack() as ctx:
        singles = ctx.enter_context(tc.tile_pool(name="singles", bufs=1))
        cw_pool = ctx.enter_context(tc.tile_pool(name="cw", bufs=BPC))
        weff_pool = ctx.enter_context(tc.tile_pool(name="weff", bufs=3))
        pa_pool = ctx.enter_context(tc.tile_pool(name="pa", bufs=12))
        out_pool = ctx.enter_context(tc.tile_pool(name="outsb", bufs=4))
        psum_pre = ctx.enter_context(
            tc.tile_pool(name="psum_pre", bufs=1, space="PSUM")
        )
        psum_mm2 = ctx.enter_context(
            tc.tile_pool(name="psum_mm2", bufs=2, space="PSUM")
        )
        psum_weff = ctx.enter_context(
            tc.tile_pool(name="psum_weff", bufs=1, space="PSUM")
        )
        psum_main = ctx.enter_context(
            tc.tile_pool(name="psum_main", bufs=4, space="PSUM")
        )

        # ---- staging.  Scalar's queue head carries the ACT table load,
        # so the latency-critical first weight DMAs go on sync/gpsimd;
        # engines round-robin between queue rows at packet granularity.
        rep_ap = rep_h.ap()
        # per-(group, chunk) tiles: finer deps, group-0 matmuls start early
        pa_tiles = [
            [
                pa_pool.tile([96, PCH], BF16, tag="pa", name=f"pa_g{g}c{c}")
                for c in range(T // PCH)
            ]
            for g in range(len(GROUPS))
        ]

        # qt_sb[p, (dc, b)] = qT[128*dc + p, b]  (small, needed first)
        qt_sb = singles.tile([128, 8 * BPC], BF16)
        nc.sync.dma_start(qt_sb[:], qT_h.ap())
        # w1t chunks as separate per-d-chunk tiles: each mm1 matmul starts
        # as soon as its own 256-column chunk lands
        w1t_tiles = []
        for ch in range(8):
            w1c = singles.tile([128, H], BF16, name=f"w1c{ch}")
            eng = nc.sync if ch % 2 == 0 else nc.gpsimd
            eng.dma_start(w1c[:], w1t_h.ap()[:, H * ch : H * ch + H])
            w1t_tiles.append(w1c)
        # w2t chunks likewise; chunk (2*hc + nh) is exactly one mm2 operand
        w2t_tiles = []
        for ch in range(4):
            w2c = singles.tile([128, 496], BF16, name=f"w2c{ch}")
            eng = nc.sync if ch % 2 == 0 else nc.gpsimd
            eng.dma_start(w2c[:], w2t_h.ap()[:, 496 * ch : 496 * ch + 496])
            w2t_tiles.append(w2c)
        wfct_sb = singles.tile([C, O], F32)
        nc.gpsimd.dma_start(wfct_sb[:], wfct_h.ap())
        b1_sb = singles.tile([128, 2], F32)
        nc.scalar.dma_start(b1_sb[:], b1_h.ap())
        b2_sb = singles.tile([1, C * K], BF16)
        nc.scalar.dma_start(b2_sb[:], b2_h.ap().unsqueeze(0))

        # replica groups in 2KB-run column chunks on the sync ring
        for gi, (b0, cnt) in enumerate(GROUPS):
            for ch in range(T // PCH):
                nc.sync.dma_start(
                    pa_tiles[gi][ch][0 : 32 * cnt, :],
                    rep_ap[gi, 0 : 32 * cnt, PCH * ch : PCH * ch + PCH],
                )
        ones_f32 = singles.tile([1, BPC], F32)
        nc.gpsimd.memset(ones_f32[:], 1.0)
        ones_sb = singles.tile([1, BPC], BF16)
        nc.vector.tensor_copy(ones_sb[:], ones_f32[:])
        ident_sb = singles.tile([BPC, BPC], F32)
        make_identity(nc, ident_sb[:])

        # weff tiles pre-created so their bfc rows {31, 63, 95} can be
        # DMA'd at staging time (no data deps; pairs with the replica
        # ones row).  gpsimd's queue is idle after its staging share.
        weff_tiles = []
        for gi, (b0, cnt) in enumerate(GROUPS):
            wg = weff_pool.tile([96, O], BF16, tag="weff", name=f"wg{gi}")
            for i in range(cnt):
                nc.gpsimd.dma_start(
                    wg[32 * i + K : 32 * i + KB, :], bfcr_h.ap()[i : i + 1, :]
                )
            weff_tiles.append(wg)

        # ---- hypernet mm1 (wide-N orientation): h[b, j] --------------
        # h[b, j] = sum_d qT[d, b] W1T[d, j]
        ph = psum_pre.tile([BPC, H], F32, tag="pre")
        for dc in range(8):
            nc.tensor.matmul(
                ph[:],
                lhsT=qt_sb[:, BPC * dc : BPC * dc + BPC],
                rhs=w1t_tiles[dc][:],
                start=(dc == 0),
                stop=(dc == 7),
            )
        # mm2 bias rows land in their psum banks first: they only need
        # staged constants, so they fill the mm1 -> transpose bubble
        HALF = C * K // 2  # 496
        pc_tiles = []
        for nh in range(2):
            pc = psum_mm2.tile([BPC, HALF], F32, tag="mm2")
            nc.tensor.matmul(
                pc[:],
                lhsT=ones_sb[:],
                rhs=b2_sb[:, HALF * nh : HALF * nh + HALF],
                start=True,
                stop=False,
            )
            pc_tiles.append(pc)
        h_sb = singles.tile([BPC, H], F32)
        # b1 is applied with tanh after the transpose (bias varies along
        # the free dim in this layout), so copy raw here.
        nc.vector.tensor_copy(h_sb[:], ph[:])

        # transpose h -> hT chunks [128 j, BPC] and apply tanh(+b1) there
        htr_sb = singles.tile([128, 2 * BPC], BF16)
        for jc in range(2):
            pt = psum_pre.tile([128, BPC], F32, tag="pre")
            nc.tensor.transpose(
                pt[:], h_sb[:, 128 * jc : 128 * jc + 128], ident_sb[:]
            )
            nc.scalar.activation(
                htr_sb[:, BPC * jc : BPC * jc + BPC], pt[:], AF.Tanh,
                bias=b1_sb[:, jc : jc + 1],
            )

        # ---- hypernet mm2: cwB[b, (c k)] = sum_h W2T[h, ck] hT[h, b] + b2 --
        cwB_sb = singles.tile([BPC, C * K], F32)
        for nh in range(2):
            pc = pc_tiles[nh]
            for hc in range(2):
                nc.tensor.matmul(
                    pc[:],
                    lhsT=htr_sb[:, BPC * hc : BPC * hc + BPC],
                    rhs=w2t_tiles[2 * hc + nh][:],
                    start=False,
                    stop=(hc == 1),
                )
            dst = cwB_sb[:, HALF * nh : HALF * nh + HALF]
            if nh == 0:
                nc.vector.tensor_copy(dst, pc[:])
            else:
                nc.scalar.activation(dst, pc[:], AF.Identity)

        # ---- per-group pipeline: cw gather -> Weff matmul -> bf16 copy.
        # cw_b[c, k] <- cwB[b, 31c + k] via small sbuf->sbuf shuffles, a
        # group's dispatches on three different queues in parallel, so
        # group 0's main matmuls start while later groups still gather.
        cw_engs = [nc.sync, nc.scalar, nc.gpsimd]
        for gi, (b0, cnt) in enumerate(GROUPS):
            cw_tiles = []
            for i in range(cnt):
                cwt = cw_pool.tile([C, K], F32, tag="cwt")
                cw_engs[i % 3].dma_start(
                    cwt[:],
                    cwB_sb[b0 + i : b0 + i + 1, :].rearrange(
                        "p (c k) -> p c k", c=C
                    ),
                )
                cw_tiles.append(cwt)
            # WeffT_b[k, o] = sum_c cw_b[c, k] WfcT[c, o]; sample i of a
            # group lives at partition base 32*i
            pw = psum_weff.tile([96, O], F32, tag="pweff")
            wg = weff_tiles[gi]
            for i in range(cnt):
                nc.tensor.matmul(
                    pw[32 * i : 32 * i + K, :],
                    lhsT=cw_tiles[i][:],
                    rhs=wfct_sb[:],
                    start=True,
                    stop=True,
                )
            for i in range(cnt):
                nc.vector.tensor_copy(
                    wg[32 * i : 32 * i + K, :], pw[32 * i : 32 * i + K, :]
                )

        # ---- main loop: keep the PE stream gap-free -------------------
        idx = 0
        out_ap = out_h.ap()
        for gi, (b0, cnt) in enumerate(GROUPS):
            wg = weff_tiles[gi]
            for i in range(cnt):
                lhsT = wg[32 * i : 32 * i + KB, :]
                b = b0 + i
                for oc in range(T // OCH):
                    osb = out_pool.tile([O, OCH], BF16, tag="osb")
                    for q in range(OCH // TCH):
                        tcn = oc * (OCH // TCH) + q
                        pm = psum_main.tile([O, TCH], F32, tag="pmm")
                        nc.tensor.matmul(
                            pm[:],
                            lhsT=lhsT,
                            rhs=pa_tiles[gi][tcn // (PCH // TCH)][
                                32 * i : 32 * i + KB,
                                TCH * (tcn % (PCH // TCH)) :
                                TCH * (tcn % (PCH // TCH)) + TCH,
                            ],
                            start=True,
                            stop=True,
                        )
                        # psum -> sbuf bf16 narrowing copy (bias already in);
                        # only DVE and ACT can read PSUM -- alternate them
                        dst = osb[:, TCH * q : TCH * q + TCH]
                        if idx % 2 == 0:
                            nc.vector.tensor_copy(dst, pm[:])
                        else:
                            nc.scalar.activation(dst, pm[:], AF.Identity)
                        idx += 1
                    nc.sync.dma_start(
                        out_ap[b, :, OCH * oc : OCH * oc + OCH], osb[:]
                    )


def get_nc(use_f32r=True):
    # use_f32r kept for test-harness compat; the data path is bf16.
    if "nc" not in _CACHED:
        _CACHED["nc"] = _build_nc()
    return _CACHED["nc"]


def make_in_maps(query, prev_attn, W1, b1, W2, b2, Wfc, bfc):
    """Shard + lay out host inputs for the 8 cores."""
    f = np.float32
    w1t = np.asarray(W1, f).T  # [D, H]
    w2t = np.asarray(W2, f).T  # [H, C*K]
    wfct = np.ascontiguousarray(np.asarray(Wfc, f).T)  # [C, O]
    b1 = np.asarray(b1, f)
    b2 = np.ascontiguousarray(np.asarray(b2, f)).astype(BF16NP)
    bfcr = np.ascontiguousarray(
        np.broadcast_to(np.asarray(bfc, f).reshape(1, O), (3, O))
    ).astype(BF16NP)
    query = np.asarray(query, f)
    prev_attn = np.asarray(prev_attn, f)

    # prepack into the SBUF partition-major images the kernel DMAs verbatim
    # w1tp[p, (dc, j)] = W1T[128*dc + p, j]
    w1tp = np.ascontiguousarray(
        w1t.reshape(8, 128, H).transpose(1, 0, 2).reshape(128, 8 * H)
    ).astype(BF16NP)
    w2tp = np.ascontiguousarray(
        w2t.reshape(2, 128, C * K).transpose(1, 0, 2).reshape(128, 2 * C * K)
    ).astype(BF16NP)
    b1p = np.ascontiguousarray(b1.reshape(2, 128).T)  # [128, 2]

    in_maps = []
    for i in range(NCORES):
        sl = slice(i * BPC, (i + 1) * BPC)
        qT = query[sl].T  # [D, BPC]
        qtp = np.ascontiguousarray(
            qT.reshape(8, 128, BPC).transpose(1, 0, 2).reshape(128, 8 * BPC)
        ).astype(BF16NP)
        # shifted replicas: paRep[g, 32*i + k, t] = pad(prev_attn)[b0+i, k+t]
        # with row 32*i + 31 = ones (pairs with the bfc row in Weff)
        padded = np.zeros((BPC, T + 2 * PAD), f)
        padded[:, PAD : PAD + T] = prev_attn[sl]
        win = np.lib.stride_tricks.sliding_window_view(padded, T, axis=1)
        # win[b, k, t] = padded[b, k + t], k in [0, 31)
        rep = np.zeros((len(GROUPS), 96, T), BF16NP)
        for g, (b0, cnt) in enumerate(GROUPS):
            for j in range(cnt):
                rep[g, 32 * j : 32 * j + K] = win[b0 + j].astype(BF16NP)
                rep[g, 32 * j + K] = BF16NP(1.0)
        in_maps.append(
            {
                "qtp": qtp,
                "paRep": rep,
                "w1tp": w1tp,
                "b1p": b1p,
                "w2tp": w2tp,
                "b2": b2,
                "wfct": wfct,
                "bfcr": bfcr,
            }
        )
    return in_maps


def assemble_output(results):
    """[8 cores] x [BPC, O, T] bf16 -> [B, T, O] f32 view."""
    full = np.concatenate(
        [r["out"].astype(np.float32) for r in results], axis=0
    )  # [B, O, T]
    return full.transpose(0, 2, 1)


def kernel(query, prev_attn, W1, b1, W2, b2, Wfc, bfc):
    nc = get_nc()
    in_maps = make_in_maps(query, prev_attn, W1, b1, W2, b2, Wfc, bfc)
    res = run_bass_kernel_spmd(nc, in_maps, list(range(NCORES)))
    return assemble_output(res.results)


# revision 19
# speedup vs baseline: 1.0462x; 1.0462x over previous
"""DynamicFilter Trainium2 kernel.

Computation (per sample b):
    h  = tanh(query @ W1.T + b1)                      [B, 256]
    cw = (h @ W2.T + b2).reshape(B, C=32, K=31)       per-sample conv weights
    x[b,t,c] = sum_k cw[b,c,k] * pad(prev_attn)[b, t+k]
    out[b,t,o] = sum_c Wfc[o,c] x[b,t,c] + bfc[o]

Key algebraic fusion: fold the fc into the conv,
    Weff[b,o,k] = sum_c Wfc[o,c] cw[b,c,k]            [B, 128, 31]
    out[b,t,o]  = sum_k Weff[b,o,k] pad(prev_attn)[b, t+k] + bfc[o]
so the T-sized work is ONE matmul per (sample, 512-wide t-chunk):
    psum[128 o, 512 t] = WeffT_b[32 k, 128 o].T @ windows[32 k, 512 t]
with the windows operand streamed from SBUF tiles holding 31 shifted
replicas of each padded row plus a row of ones; the matching 32nd
stationary row holds bfc, so PSUM accumulates conv + bias exactly in
fp32 and the psum->sbuf drain is a plain dtype-narrowing copy.

The whole T-sized data path runs in bf16 (the correctness gate is
rel_err < 2e-2; bf16 rounding costs ~4e-3): replicas, matmul operands
and the output stream are all bf16, halving HBM traffic -- the f32
profile showed all 16 SDMA engines pegged at the ~350 GB/s HBM
roofline.  PSUM accumulation stays fp32.

Main-loop discipline: the PE p-state ramps to 2.4 GHz only after ~3us
of CONTINUOUS execution (it idles at 1.2 GHz otherwise), so the 64
conv matmuls are kept gap-free: 5 PSUM banks round-robin, the drain
copies rotate over vector/scalar/gpsimd so no single engine falls
behind, and all out-DMA dispatches live on the sync queue.

Sharding: data-parallel over batch. 64 samples / 8 cores = 8 per core.
Weights replicated. Output written bf16 [b, o, t] in [128, 2048] tiles
(4 KB contiguous runs); host upcasts and returns a transposed view
[B, T, O] in f32.
"""

import sys

import numpy as np

if "/opt/trn_rl_repo" not in sys.path:
    sys.path.insert(0, "/opt/trn_rl_repo")

from contextlib import ExitStack

import ml_dtypes

import concourse.bass as bass
import concourse.mybir as mybir
import concourse.tile as tile
from concourse import bacc
from concourse.ap import AP
from concourse.bass_utils import run_bass_kernel_spmd
from concourse.masks import make_identity

# Problem shapes (hardcoded per contract).
B, T = 64, 4096
D, H = 1024, 256
C, K, O = 32, 31, 128
KB = K + 1  # conv taps + the folded-bias ones row
PAD = (K - 1) // 2  # 15
NCORES = 8
BPC = B // NCORES  # 8 samples per core
TCH = 512  # t-chunk (matmul moving free dim, one PSUM bank)
NT = T // TCH  # 8 chunks per sample
OCH = 2048  # out-tile column width (4 KB bf16 runs)
PCH = 1024  # replica chunk width (2 KB bf16 runs)
GROUPS = [(0, 3), (3, 3), (6, 2)]  # (first sample, count) per replica tile

F32 = mybir.dt.float32
BF16 = mybir.dt.bfloat16
AF = mybir.ActivationFunctionType
BF16NP = ml_dtypes.bfloat16

_CACHED = {}


def _build_nc():
    nc = bacc.Bacc(
        "TRN2", target_bir_lowering=False, debug=False, num_devices=NCORES
    )

    # host-prepacked layouts: single contiguous DMAs into the exact SBUF
    # images (descriptor count on the HWDGE rings is a scarce resource)
    qT_h = nc.dram_tensor("qtp", [128, 8 * BPC], BF16, kind="ExternalInput")
    rep_h = nc.dram_tensor("paRep", [len(GROUPS), 96, T], BF16,
                           kind="ExternalInput")
    w1t_h = nc.dram_tensor("w1tp", [128, 8 * H], BF16, kind="ExternalInput")
    b1_h = nc.dram_tensor("b1p", [128, 2], F32, kind="ExternalInput")
    w2t_h = nc.dram_tensor("w2tp", [128, 2 * C * K], BF16,
                           kind="ExternalInput")
    b2_h = nc.dram_tensor("b2", [C * K], BF16, kind="ExternalInput")
    wfct_h = nc.dram_tensor("wfct", [C, O], F32, kind="ExternalInput")
    bfcr_h = nc.dram_tensor("bfcr", [3, O], BF16, kind="ExternalInput")
    out_h = nc.dram_tensor("out", [BPC, O, T], BF16, kind="ExternalOutput")

    with tile.TileContext(nc) as tc:
        _emit(tc, qT_h, rep_h, w1t_h, b1_h, w2t_h, b2_h, wfct_h, bfcr_h, out_h)

    nc.compile()
    return nc


def _emit(tc, qT_h, rep_h, w1t_h, b1_h, w2t_h, b2_h, wfct_h, bfcr_h, out_h):
    nc = tc.nc
    with ExitStack() as ctx:
        singles = ctx.enter_context(tc.tile_pool(name="singles", bufs=1))
        cw_pool = ctx.enter_context(tc.tile_pool(name="cw", bufs=BPC))
        weff_pool = ctx.enter_context(tc.tile_pool(name="weff", bufs=3))
        pa_pool = ctx.enter_context(tc.tile_pool(name="pa", bufs=12))
        out_pool = ctx.enter_context(tc.tile_pool(name="outsb", bufs=4))
        # one bank shared by the pre-chain (ph/pt, done by ~15us) and the
        # weff tiles (first used after), so the main loop keeps 5 banks
        psum_pre = ctx.enter_context(
            tc.tile_pool(name="psum_pre", bufs=1, space="PSUM")
        )
        psum_mm2 = ctx.enter_context(
            tc.tile_pool(name="psum_mm2", bufs=2, space="PSUM")
        )
        psum_weff = psum_pre
        psum_main = ctx.enter_context(
            tc.tile_pool(name="psum_main", bufs=5, space="PSUM")
        )

        # ---- staging.  Scalar's queue head carries the ACT table load,
        # so the latency-critical first weight DMAs go on sync/gpsimd;
        # engines round-robin between queue rows at packet granularity.
        rep_ap = rep_h.ap()
        # per-(group, chunk) tiles: finer deps, group-0 matmuls start early
        pa_tiles = [
            [
                pa_pool.tile([96, PCH], BF16, tag="pa", name=f"pa_g{g}c{c}")
                for c in range(T // PCH)
            ]
            for g in range(len(GROUPS))
        ]

        # qt_sb[p, (dc, b)] = qT[128*dc + p, b]  (small, needed first)
        qt_sb = singles.tile([128, 8 * BPC], BF16)
        nc.sync.dma_start(qt_sb[:], qT_h.ap())
        # w1t chunks as separate per-d-chunk tiles: each mm1 matmul starts
        # as soon as its own 256-column chunk lands
        w1t_tiles = []
        for ch in range(8):
            w1c = singles.tile([128, H], BF16, name=f"w1c{ch}")
            eng = nc.sync if ch % 2 == 0 else nc.gpsimd
            eng.dma_start(w1c[:], w1t_h.ap()[:, H * ch : H * ch + H])
            w1t_tiles.append(w1c)
        # w2t chunks likewise; chunk (2*hc + nh) is exactly one mm2 operand
        w2t_tiles = []
        for ch in range(4):
            w2c = singles.tile([128, 496], BF16, name=f"w2c{ch}")
            eng = nc.sync if ch % 2 == 0 else nc.gpsimd
            eng.dma_start(w2c[:], w2t_h.ap()[:, 496 * ch : 496 * ch + 496])
            w2t_tiles.append(w2c)
        wfct_sb = singles.tile([C, O], F32)
        nc.gpsimd.dma_start(wfct_sb[:], wfct_h.ap())
        b1_sb = singles.tile([128, 2], F32)
        nc.scalar.dma_start(b1_sb[:], b1_h.ap())
        b2_sb = singles.tile([1, C * K], BF16)
        nc.scalar.dma_start(b2_sb[:], b2_h.ap().unsqueeze(0))

        # replica groups in 2KB-run column chunks on the sync ring
        for gi, (b0, cnt) in enumerate(GROUPS):
            for ch in range(T // PCH):
                nc.sync.dma_start(
                    pa_tiles[gi][ch][0 : 32 * cnt, :],
                    rep_ap[gi, 0 : 32 * cnt, PCH * ch : PCH * ch + PCH],
                )
        ones_f32 = singles.tile([1, BPC], F32)
        nc.gpsimd.memset(ones_f32[:], 1.0)
        ones_sb = singles.tile([1, BPC], BF16)
        nc.vector.tensor_copy(ones_sb[:], ones_f32[:])
        ident_sb = singles.tile([BPC, BPC], F32)
        make_identity(nc, ident_sb[:])

        # weff tiles pre-created so their bfc rows {31, 63, 95} can be
        # DMA'd at staging time (no data deps; pairs with the replica
        # ones row).  gpsimd's queue is idle after its staging share.
        weff_tiles = []
        for gi, (b0, cnt) in enumerate(GROUPS):
            wg = weff_pool.tile([96, O], BF16, tag="weff", name=f"wg{gi}")
            for i in range(cnt):
                nc.gpsimd.dma_start(
                    wg[32 * i + K : 32 * i + KB, :], bfcr_h.ap()[i : i + 1, :]
                )
            weff_tiles.append(wg)

        # ---- hypernet mm1 (wide-N orientation): h[b, j] --------------
        # h[b, j] = sum_d qT[d, b] W1T[d, j]
        ph = psum_pre.tile([BPC, H], F32, tag="pre")
        for dc in range(8):
            nc.tensor.matmul(
                ph[:],
                lhsT=qt_sb[:, BPC * dc : BPC * dc + BPC],
                rhs=w1t_tiles[dc][:],
                start=(dc == 0),
                stop=(dc == 7),
            )
        # mm2 bias rows land in their psum banks first: they only need
        # staged constants, so they fill the mm1 -> transpose bubble
        HALF = C * K // 2  # 496
        pc_tiles = []
        for nh in range(2):
            pc = psum_mm2.tile([BPC, HALF], F32, tag="mm2")
            nc.tensor.matmul(
                pc[:],
                lhsT=ones_sb[:],
                rhs=b2_sb[:, HALF * nh : HALF * nh + HALF],
                start=True,
                stop=False,
            )
            pc_tiles.append(pc)
        h_sb = singles.tile([BPC, H], F32)
        # b1 is applied with tanh after the transpose (bias varies along
        # the free dim in this layout), so copy raw here.
        nc.vector.tensor_copy(h_sb[:], ph[:])

        # transpose h -> hT chunks [128 j, BPC] and apply tanh(+b1) there
        htr_sb = singles.tile([128, 2 * BPC], BF16)
        for jc in range(2):
            pt = psum_pre.tile([128, BPC], F32, tag="pre")
            nc.tensor.transpose(
                pt[:], h_sb[:, 128 * jc : 128 * jc + 128], ident_sb[:]
            )
            nc.scalar.activation(
                htr_sb[:, BPC * jc : BPC * jc + BPC], pt[:], AF.Tanh,
                bias=b1_sb[:, jc : jc + 1],
            )

        # ---- hypernet mm2: cwB[b, (c k)] = sum_h W2T[h, ck] hT[h, b] + b2 --
        cwB_sb = singles.tile([BPC, C * K], F32)
        for nh in range(2):
            pc = pc_tiles[nh]
            for hc in range(2):
                nc.tensor.matmul(
                    pc[:],
                    lhsT=htr_sb[:, BPC * hc : BPC * hc + BPC],
                    rhs=w2t_tiles[2 * hc + nh][:],
                    start=False,
                    stop=(hc == 1),
                )
            dst = cwB_sb[:, HALF * nh : HALF * nh + HALF]
            if nh == 0:
                nc.vector.tensor_copy(dst, pc[:])
            else:
                nc.scalar.activation(dst, pc[:], AF.Identity)

        # ---- per-group pipeline: cw gather -> Weff matmul -> bf16 copy.
        # cw_b[c, k] <- cwB[b, 31c + k] via small sbuf->sbuf shuffles, a
        # group's dispatches on three different queues in parallel, so
        # group 0's main matmuls start while later groups still gather.
        # group 0's gathers go on queues that are idle by mm2-end (sync is
        # still working through its replica dispatch backlog then)
        cw_engs_by_group = [
            [nc.scalar, nc.gpsimd, nc.scalar],
            [nc.sync, nc.scalar, nc.gpsimd],
            [nc.sync, nc.scalar, nc.gpsimd],
        ]
        for gi, (b0, cnt) in enumerate(GROUPS):
            cw_engs = cw_engs_by_group[gi]
            cw_tiles = []
            for i in range(cnt):
                cwt = cw_pool.tile([C, K], F32, tag="cwt")
                cw_engs[i % 3].dma_start(
                    cwt[:],
                    cwB_sb[b0 + i : b0 + i + 1, :].rearrange(
                        "p (c k) -> p c k", c=C
                    ),
                )
                cw_tiles.append(cwt)
            # WeffT_b[k, o] = sum_c cw_b[c, k] WfcT[c, o]; sample i of a
            # group lives at partition base 32*i
            pw = psum_weff.tile([96, O], F32, tag="pre")
            wg = weff_tiles[gi]
            for i in range(cnt):
                nc.tensor.matmul(
                    pw[32 * i : 32 * i + K, :],
                    lhsT=cw_tiles[i][:],
                    rhs=wfct_sb[:],
                    start=True,
                    stop=True,
                )
            for i in range(cnt):
                nc.vector.tensor_copy(
                    wg[32 * i : 32 * i + K, :], pw[32 * i : 32 * i + K, :]
                )

        # ---- main loop: keep the PE stream gap-free -------------------
        idx = 0
        out_ap = out_h.ap()
        for gi, (b0, cnt) in enumerate(GROUPS):
            wg = weff_tiles[gi]
            for i in range(cnt):
                lhsT = wg[32 * i : 32 * i + KB, :]
                b = b0 + i
                for oc in range(T // OCH):
                    osb = out_pool.tile([O, OCH], BF16, tag="osb")
                    for q in range(OCH // TCH):
                        tcn = oc * (OCH // TCH) + q
                        pm = psum_main.tile([O, TCH], F32, tag="pmm")
                        nc.tensor.matmul(
                            pm[:],
                            lhsT=lhsT,
                            rhs=pa_tiles[gi][tcn // (PCH // TCH)][
                                32 * i : 32 * i + KB,
                                TCH * (tcn % (PCH // TCH)) :
                                TCH * (tcn % (PCH // TCH)) + TCH,
                            ],
                            start=True,
                            stop=True,
                        )
                        # psum -> sbuf bf16 narrowing copy (bias already in);
                        # only DVE and ACT can read PSUM -- alternate them
                        dst = osb[:, TCH * q : TCH * q + TCH]
                        if idx % 2 == 0:
                            nc.vector.tensor_copy(dst, pm[:])
                        else:
                            nc.scalar.activation(dst, pm[:], AF.Identity)
                        idx += 1
                    nc.sync.dma_start(
                        out_ap[b, :, OCH * oc : OCH * oc + OCH], osb[:]
                    )


def get_nc(use_f32r=True):
    # use_f32r kept for test-harness compat; the data path is bf16.
    if "nc" not in _CACHED:
        _CACHED["nc"] = _build_nc()
    return _CACHED["nc"]


def make_in_maps(query, prev_attn, W1, b1, W2, b2, Wfc, bfc):
    """Shard + lay out host inputs for the 8 cores."""
    f = np.float32
    w1t = np.asarray(W1, f).T  # [D, H]
    w2t = np.asarray(W2, f).T  # [H, C*K]
    wfct = np.ascontiguousarray(np.asarray(Wfc, f).T)  # [C, O]
    b1 = np.asarray(b1, f)
    b2 = np.ascontiguousarray(np.asarray(b2, f)).astype(BF16NP)
    bfcr = np.ascontiguousarray(
        np.broadcast_to(np.asarray(bfc, f).reshape(1, O), (3, O))
    ).astype(BF16NP)
    query = np.asarray(query, f)
    prev_attn = np.asarray(prev_attn, f)

    # prepack into the SBUF partition-major images the kernel DMAs verbatim
    # w1tp[p, (dc, j)] = W1T[128*dc + p, j]
    w1tp = np.ascontiguousarray(
        w1t.reshape(8, 128, H).transpose(1, 0, 2).reshape(128, 8 * H)
    ).astype(BF16NP)
    w2tp = np.ascontiguousarray(
        w2t.reshape(2, 128, C * K).transpose(1, 0, 2).reshape(128, 2 * C * K)
    ).astype(BF16NP)
    b1p = np.ascontiguousarray(b1.reshape(2, 128).T)  # [128, 2]

    in_maps = []
    for i in range(NCORES):
        sl = slice(i * BPC, (i + 1) * BPC)
        qT = query[sl].T  # [D, BPC]
        qtp = np.ascontiguousarray(
            qT.reshape(8, 128, BPC).transpose(1, 0, 2).reshape(128, 8 * BPC)
        ).astype(BF16NP)
        # shifted replicas: paRep[g, 32*i + k, t] = pad(prev_attn)[b0+i, k+t]
        # with row 32*i + 31 = ones (pairs with the bfc row in Weff)
        padded = np.zeros((BPC, T + 2 * PAD), f)
        padded[:, PAD : PAD + T] = prev_attn[sl]
        win = np.lib.stride_tricks.sliding_window_view(padded, T, axis=1)
        # win[b, k, t] = padded[b, k + t], k in [0, 31)
        rep = np.zeros((len(GROUPS), 96, T), BF16NP)
        for g, (b0, cnt) in enumerate(GROUPS):
            for j in range(cnt):
                rep[g, 32 * j : 32 * j + K] = win[b0 + j].astype(BF16NP)
                rep[g, 32 * j + K] = BF16NP(1.0)
        in_maps.append(
            {
                "qtp": qtp,
                "paRep": rep,
                "w1tp": w1tp,
                "b1p": b1p,
                "w2tp": w2tp,
                "b2": b2,
                "wfct": wfct,
                "bfcr": bfcr,
            }
        )
    return in_maps


def assemble_output(results):
    """[8 cores] x [BPC, O, T] bf16 -> [B, T, O] f32 view."""
    full = np.concatenate(
        [r["out"].astype(np.float32) for r in results], axis=0
    )  # [B, O, T]
    return full.transpose(0, 2, 1)


def kernel(query, prev_attn, W1, b1, W2, b2, Wfc, bfc):
    nc = get_nc()
    in_maps = make_in_maps(query, prev_attn, W1, b1, W2, b2, Wfc, bfc)
    res = run_bass_kernel_spmd(nc, in_maps, list(range(NCORES)))
    return assemble_output(res.results)
